# revision 1
# baseline (speedup 1.0000x reference)
"""BiLSTM encoder Bass/Tile kernel for TRN2.

Design (per core, uniform SPMD program, data-parallel):
 - cores 0-3: forward direction, batch slices of 8; cores 4-7: backward
   (host pre-reverses the backward input, so the device program is uniform).
 - L=2 stacked LSTM layers, software-pipelined: within each chunk-loop
   iteration, layer-0 steps of chunk c and layer-1 steps of chunk c-1 are
   interleaved so each layer's serial gate chain hides under the other
   layer's matmul stream (keeps PE busy -> HAM stays un-throttled).
 - Transposed state layout: h.T/c.T live as [128, 4*b] tiles.
 - zx (input part) precomputed per chunk by dense matmuls, fp16 weights.
 - Gate columns host-permuted to [f, i, j, o]: one merged sigmoid for f+i,
   forget bias folded into the zx PSUM->SBUF copy, c/h muls on GpSimd.
 - Masking by `lengths` and direction reversal are host-side (outputs past
   length are zeroed at the end; the unmasked recurrence is exact there).
"""

import numpy as np
from contextlib import ExitStack

import concourse.bass as bass
import concourse.bacc as bacc
import concourse.tile as tile
import concourse.mybir as mybir
from concourse.bass import ds, ts
from concourse.bass_utils import run_bass_kernel_spmd

F16 = mybir.dt.float16
F32 = mybir.dt.float32
AF = mybir.ActivationFunctionType

B, D, H, L = 32, 512, 512, 2
G = 4 * H            # 2048 gate rows
KT = H // 128        # 4 k-tiles
MT = G // 128        # 16 m-tiles
FORGET_BIAS = 1.0


def build_program(T=1024, Tc=64, b=8, n_cores=8):
    """Build and compile the SPMD program. Returns nc.

    Pipeline (lag-2): in each unrolled body for L0-chunk i, layer-1 runs
    chunk i-2, and the zx matmuls for zx0(i+1) / zx1(i-1) are spread as
    small units between recurrent steps so the PE never idles.
    """
    NCH = T // Tc
    assert T % Tc == 0 and NCH >= 4 and NCH % 2 == 0
    nc = bacc.Bacc("TRN2", target_bir_lowering=False, debug=False,
                   num_devices=n_cores)

    # xT padded by two chunks of zeros (prefetch beyond the end is garbage)
    xT_d = nc.dram_tensor("xT", [KT, 128, T + Tc, b], F16, kind="ExternalInput")
    wx_d = nc.dram_tensor("wx", [L, KT, 128, G], F16, kind="ExternalInput")
    wh_d = nc.dram_tensor("wh", [L, KT, 128, G], F16, kind="ExternalInput")
    yT_d = nc.dram_tensor("yT", [128, T, KT, b], F16, kind="ExternalOutput")

    with tile.TileContext(nc) as tc, ExitStack() as ctx:
        wpool = ctx.enter_context(tc.tile_pool(name="w", bufs=1))
        pers = ctx.enter_context(tc.tile_pool(name="pers", bufs=1))
        gates = ctx.enter_context(tc.tile_pool(name="gates", bufs=3))
        psG = ctx.enter_context(tc.tile_pool(name="psG", bufs=1, space="PSUM"))
        psX = ctx.enter_context(tc.tile_pool(name="psX", bufs=2, space="PSUM"))

        # resident weights: [128, KT, G] each (gate blocks already [f,i,j,o])
        wx_sb = [wpool.tile([128, KT, G], F16, tag=f"wx{l}", name=f"wx{l}")
                 for l in range(L)]
        wh_sb = [wpool.tile([128, KT, G], F16, tag=f"wh{l}", name=f"wh{l}")
                 for l in range(L)]
        for l in range(L):
            nc.sync.dma_start(out=wx_sb[l][:],
                              in_=wx_d[l].rearrange("k p g -> p k g"))
            nc.sync.dma_start(out=wh_sb[l][:],
                              in_=wh_d[l].rearrange("k p g -> p k g"))

        # persistent state / staging (fixed addresses, rewritten in place)
        hprev = [pers.tile([128, KT * b], F16, tag=f"h{l}", name=f"h{l}")
                 for l in range(L)]
        cT = [pers.tile([128, KT * b], F32, tag=f"c{l}", name=f"c{l}")
              for l in range(L)]
        for l in range(L):
            nc.gpsimd.memset(hprev[l][:], 0.0)
            nc.gpsimd.memset(cT[l][:], 0.0)
        xsP = [pers.tile([128, KT, Tc, b], F16, tag=f"xs{p}", name=f"xs{p}")
               for p in range(2)]
        zx0P = [pers.tile([128, Tc, MT, b], F32, tag=f"zx0{p}", name=f"zx0{p}")
                for p in range(2)]
        zx1P = [pers.tile([128, Tc, MT, b], F32, tag=f"zx1{p}", name=f"zx1{p}")
                for p in range(2)]
        st0P = [pers.tile([128, Tc, KT, b], F16, tag=f"st0{p}", name=f"st0{p}")
                for p in range(2)]
        st16_1 = pers.tile([128, Tc, KT, b], F16, tag="st1", name="st1")

        NCOL = Tc * b
        NN = max(1, NCOL // 512)
        NS = min(512, NCOL)
        TPC = NS // b

        def xs_load(p, t0):
            nc.sync.dma_start(
                out=xsP[p][:],
                in_=xT_d[:, :, ds(t0, Tc), :].rearrange("k p t b -> p k t b"))

        def zx_units(zx_t, lhsT, rhs_k):
            """List of closures; each emits 4 accum MMs + 1 copy for (m, n).
            m 0..3 is the f gate: fold in the forget bias during the copy."""
            def unit(m, n):
                def emit():
                    ps = psX.tile([128, TPC, b], F32, tag="psx", name="psx")
                    for k in range(KT):
                        nc.tensor.matmul(
                            ps[:],
                            lhsT=lhsT[:, k, m * 128:(m + 1) * 128],
                            rhs=rhs_k(k)[:, n * TPC:(n + 1) * TPC, :],
                            start=(k == 0), stop=(k == KT - 1))
                    dst = zx_t[:, n * TPC:(n + 1) * TPC, m, :]
                    if m < 4:
                        nc.vector.tensor_scalar_add(dst, ps[:], FORGET_BIAS)
                    else:
                        nc.vector.tensor_copy(dst, ps[:])
                return emit
            return [unit(m, n) for m in range(MT) for n in range(NN)]

        def interleave(ua, ub):
            out = []
            for i in range(max(len(ua), len(ub))):
                if i < len(ua):
                    out.append(ua[i])
                if i < len(ub):
                    out.append(ub[i])
            return out

        def step(l, tl, zx_t, st16):
            """One recurrent step. Gate blocks: m0-3=f, 4-7=i, 8-11=j, 12-15=o."""
            gb = 4 * b
            if tl == 0:
                hsrc = lambda k: hprev[l][:, k * b:(k + 1) * b]
            else:
                hsrc = lambda k: st16[:, tl - 1, k, :]
            pzfi = psG.tile([128, 2 * gb], F32, tag=f"pzfi{l}", name=f"pzfi{l}")
            pzj = psG.tile([128, gb], F32, tag=f"pzj{l}", name=f"pzj{l}")
            pzo = psG.tile([128, gb], F32, tag=f"pzo{l}", name=f"pzo{l}")

            def pzdst(m):
                if m < 8:
                    return pzfi[:, m * b:(m + 1) * b]
                if m < 12:
                    return pzj[:, (m - 8) * b:(m - 7) * b]
                return pzo[:, (m - 12) * b:(m - 11) * b]

            for m in range(MT):
                for k in range(KT):
                    nc.tensor.matmul(
                        pzdst(m),
                        lhsT=wh_sb[l][:, k, m * 128:(m + 1) * 128],
                        rhs=hsrc(k),
                        start=(k == 0), stop=(k == KT - 1))

            zs = gates.tile([128, MT * b], F32, tag=f"zs{l}", name=f"zs{l}")
            gfi = gates.tile([128, 2 * gb], F32, tag=f"gfi{l}", name=f"gfi{l}")
            gj = gates.tile([128, gb], F32, tag=f"gj{l}", name=f"gj{l}")
            go = gates.tile([128, gb], F32, tag=f"go{l}", name=f"go{l}")
            t1 = gates.tile([128, gb], F32, tag=f"t1{l}", name=f"t1{l}")
            tch = gates.tile([128, gb], F32, tag=f"tch{l}", name=f"tch{l}")
            # DVE: adds + t1 + cadd + hmul; GP: cmul; ACT: 4 ops
            nc.vector.tensor_add(zs[:, 0:2 * gb], pzfi[:], zx_t[:, tl, 0:8, :])
            nc.scalar.activation(gfi[:], zs[:, 0:2 * gb], AF.Sigmoid)
            nc.gpsimd.tensor_mul(cT[l][:], gfi[:, 0:gb], cT[l][:])
            nc.vector.tensor_add(zs[:, 2 * gb:3 * gb], pzj[:],
                                 zx_t[:, tl, 8:12, :])
            nc.scalar.activation(gj[:], zs[:, 2 * gb:3 * gb], AF.Tanh)
            nc.vector.tensor_add(zs[:, 3 * gb:4 * gb], pzo[:],
                                 zx_t[:, tl, 12:16, :])
            nc.scalar.activation(go[:], zs[:, 3 * gb:4 * gb], AF.Sigmoid)
            nc.vector.tensor_mul(t1[:], gfi[:, gb:2 * gb], gj[:])
            nc.vector.tensor_add(cT[l][:], cT[l][:], t1[:])
            nc.scalar.activation(tch[:], cT[l][:], AF.Tanh)
            nc.vector.tensor_mul(st16[:, tl, :, :], go[:], tch[:])

        def carry_h(l, st16):
            nc.vector.tensor_copy(hprev[l][:], st16[:, Tc - 1, :, :])

        def rec_chunk(l, zx_t, st16, units):
            """Tc steps of one layer with zx units spread between steps."""
            done = 0
            for tl in range(Tc):
                step(l, tl, zx_t, st16)
                want = (tl + 1) * len(units) // Tc
                while done < want:
                    units[done]()
                    done += 1
            carry_h(l, st16)

        def rec_pair(zx_l0, st0, zx_l1, units):
            """Tc interleaved L0/L1 steps with zx units spread in."""
            done = 0
            for tl in range(Tc):
                step(0, tl, zx_l0, st0)
                want = (2 * tl + 1) * len(units) // (2 * Tc)
                while done < want:
                    units[done]()
                    done += 1
                step(1, tl, zx_l1, st16_1)
                want = (2 * tl + 2) * len(units) // (2 * Tc)
                while done < want:
                    units[done]()
                    done += 1
            carry_h(0, st0)
            carry_h(1, st16_1)

        st0rhs = lambda p: (lambda k: st0P[p][:, :, k, :])
        xsrhs = lambda p: (lambda k: xsP[p][:, k, :, :])

        # ---- peel: L0 chunks 0,1; prepare zx0(2), zx1(0) ----
        xs_load(0, 0)
        xs_load(1, Tc)
        for u in zx_units(zx0P[0], wx_sb[0], xsrhs(0)):
            u()
        rec_chunk(0, zx0P[0], st0P[0],
                  zx_units(zx0P[1], wx_sb[0], xsrhs(1)))
        xs_load(0, 2 * Tc)
        rec_chunk(0, zx0P[1], st0P[1],
                  interleave(zx_units(zx1P[0], wx_sb[1], st0rhs(0)),
                             zx_units(zx0P[0], wx_sb[0], xsrhs(0))))

        # ---- steady state: 7 iterations x 2 bodies (L0 chunk i, L1 i-2) ----
        with tc.For_i(0, T - 2 * Tc, 2 * Tc) as tb:
            # body A: L0 chunk i (parity 0), L1 chunk i-2 (parity 0)
            xs_load(1, tb + 3 * Tc)
            xs_load(0, tb + 4 * Tc)
            rec_pair(zx0P[0], st0P[0], zx1P[0],
                     interleave(zx_units(zx0P[1], wx_sb[0], xsrhs(1)),
                                zx_units(zx1P[1], wx_sb[1], st0rhs(1))))
            nc.sync.dma_start(out=yT_d[:, ds(tb, Tc), :, :], in_=st16_1[:])
            # body B: L0 chunk i+1 (parity 1), L1 chunk i-1 (parity 1)
            rec_pair(zx0P[1], st0P[1], zx1P[1],
                     interleave(zx_units(zx0P[0], wx_sb[0], xsrhs(0)),
                                zx_units(zx1P[0], wx_sb[1], st0rhs(0))))
            nc.sync.dma_start(out=yT_d[:, ds(tb + Tc, Tc), :, :], in_=st16_1[:])

        # ---- drain: L1 chunks NCH-2, NCH-1 ----
        rec_chunk(1, zx1P[0], st16_1,
                  zx_units(zx1P[1], wx_sb[1], st0rhs(1)))
        nc.sync.dma_start(out=yT_d[:, T - 2 * Tc:T - Tc, :, :], in_=st16_1[:])
        rec_chunk(1, zx1P[1], st16_1, [])
        nc.sync.dma_start(out=yT_d[:, T - Tc:T, :, :], in_=st16_1[:])

    nc.compile()
    return nc


# ---------------- host glue ----------------

def reverse_seq(x, lengths):
    t = np.arange(x.shape[1])[None, :]
    ln = lengths[:, None]
    idx = np.where(t < ln, ln - 1 - t, t)
    return np.take_along_axis(x, idx[:, :, None], axis=1)


def permute_gates(W):
    """[.., 4H] gate columns i,j,f,o -> f,i,j,o."""
    Wi, Wj, Wf, Wo = (W[..., 0:H], W[..., H:2 * H],
                      W[..., 2 * H:3 * H], W[..., 3 * H:4 * H])
    return np.concatenate([Wf, Wi, Wj, Wo], axis=-1)


def make_in_maps(inputs, lengths, Wf, Wb, T, b, n_cores=8):
    """Build per-core input dicts. cores 0..3 fwd, 4..7 bwd."""
    xr = reverse_seq(inputs, lengths)
    per_dir = n_cores // 2
    in_maps = []
    for c in range(n_cores):
        d = c // per_dir
        s = (c % per_dir) * b
        x = (inputs if d == 0 else xr)[s:s + b, :T]     # [b, T, D]
        W = permute_gates(np.asarray(Wf if d == 0 else Wb))
        xT = np.ascontiguousarray(x.transpose(2, 1, 0))  # [D, T, b]
        xT = xT.reshape(KT, 128, T, b).astype(np.float16)
        wx = W[:, :D].reshape(L, KT, 128, G).astype(np.float16)
        wh = W[:, D:].reshape(L, KT, 128, G).astype(np.float16)
        in_maps.append({"xT": xT, "wx": wx, "wh": wh})
    return in_maps


def assemble_output(results, lengths, T, b, n_cores=8):
    """results[c]["yT"]: [128, T, KT, b] f16 -> full [B, T, 2H] masked."""
    per_dir = n_cores // 2
    out = np.zeros((B, T, 2 * H), np.float32)
    for c in range(n_cores):
        d = c // per_dir
        s = (c % per_dir) * b
        yT = results[c]["yT"].astype(np.float32)        # [128, T, KT, b]
        y = yT.transpose(3, 1, 2, 0).reshape(b, T, H)   # h[j,t,128k+p]
        if d == 0:
            out[s:s + b, :, :H] = y
        else:
            out[s:s + b, :, H:] = reverse_seq(y, lengths[s:s + b])
    mask = (np.arange(T)[None, :] < lengths[:, None])[:, :, None]
    return np.where(mask, out, 0.0).astype(np.float32)


# ---------------- grading entry point ----------------

_NC_CACHE = {}


def kernel(inputs, lengths, Wf, bf, Wb, bb):
    """Full-input BiLSTM encoder on 8 TRN2 NeuronCores.

    inputs: [32,1024,512] f32; lengths: [32] int; Wf/Wb: [2,1024,2048] f32;
    bf/bb: [2,2048] f32 (zeros in this problem; a nonzero forget-gate-style
    constant is not supported beyond the fixed FORGET_BIAS of the reference).
    Returns [32,1024,1024] f32.
    """
    T, Tc, b = 1024, 64, 8
    inputs = np.asarray(inputs, dtype=np.float32)
    lengths = np.asarray(lengths).astype(np.int64)
    Wf = np.asarray(Wf, dtype=np.float32)
    Wb = np.asarray(Wb, dtype=np.float32)

    key = (T, Tc, b)
    if key not in _NC_CACHE:
        _NC_CACHE[key] = build_program(T=T, Tc=Tc, b=b)
    nc = _NC_CACHE[key]

    in_maps = make_in_maps(inputs, lengths, Wf, Wb, T, b, Tc_pad=Tc)
    r = run_bass_kernel_spmd(nc, in_maps, list(range(8)), trace=False)
    return assemble_output(r.results, lengths, T, b)
